# revision 1
# baseline (speedup 1.0000x reference)
"""Block-quantize kernel for Trainium2 (8 NeuronCores, data-parallel).

Reference semantics (fp32, wl=8, ebit=8):
    m  = max(max|x|, 1e-10)                      # global over all elements
    e  = clip(floor(log2(m)), -128, 127)
    y  = clip(round_half_even(x * 2^(6-e)), -128, 127) * 2^(e-6)

Implementation:
  - x (16, 2048, 4096) f32 is sharded on the batch dim: 2 batches per core
    (64 MiB), treated as a flat per-core vector so every [128, TILE_F] tile
    is one contiguous DMA.
  - Pass 1 streams the shard computing abs-max (DVE tensor_reduce with
    apply_absolute_value), reduces across partitions (GpSimd
    partition_all_reduce), then a 4-byte AllReduce(max) across the 8 cores.
  - e and the two power-of-two scales are derived with exact int32 bit
    arithmetic on the fp32 representation (all values are multiples of 2^23
    with small multipliers, so the DVE's internal fp32 math is exact):
        p  = bits(m) & 0x7F800000                 # bits of 2^e
        s2 = bits^-1(p - (6<<23))                 # 2^(e-6)
        s1 = bits^-1(((254<<23) - p) + (6<<23))   # 2^(6-e)
  - The last KEEP pass-1 tiles stay resident in SBUF (SBUF fits 12 of the
    32 tiles), so pass 2 skips re-reading 18 MiB of the 64 MiB shard:
    total HBM traffic 174 MiB/core vs the naive two-pass 192 MiB.
  - Pass 2 streams the rest of the shard again:
        r  = x*s1 + C        # C = 1.5*2^23; fp32 RNE add == round-half-even
        u  = min(r, C+127) ; max(u, C-128)        # clip in shifted domain
        y  = (u - C) * s2                         # both steps exact in fp32
    Every elementwise op is a dual-op DVE tensor_scalar (2x perf mode).
  - All four elementwise/reduce values stay exact in fp32, so the result is
    bit-identical to the reference evaluated in exact arithmetic.
"""
import sys

if "/opt/trn_rl_repo" not in sys.path:
    sys.path.insert(0, "/opt/trn_rl_repo")

import numpy as np

N_CORES = 8
B, S, D = 16, 2048, 4096          # full input shape
PB = B // N_CORES                  # batches per core
P = 128                            # SBUF partitions
NELEM = PB * S * D                 # per-core elements (16.8M, 64 MiB)
TILE_F = 4096                      # tile free dim -> [128, 4096] = 2 MiB
BUFS = 3                           # streaming-pool slots
KEEP = 9                           # pass-1 tail tiles kept in SBUF for pass 2
C_MAGIC = 12582912.0               # 1.5 * 2^23, round-to-nearest-even magic

_CACHE = {}


def _build(reps: int = 1, tile_f: int = TILE_F, bufs: int = BUFS,
           clip_engine: str = "vector", keep: int = KEEP, cc: str = "ar"):
    import concourse.mybir as mybir
    from concourse import bacc, bass_isa, tile

    DT = mybir.dt.float32
    DI = mybir.dt.int32
    A = mybir.AluOpType

    ch = P * tile_f                # elements per tile
    n_t = NELEM // ch              # tiles per pass
    assert n_t * ch == NELEM
    n_keep = min(keep, n_t - 1)    # tail tiles that stay resident in SBUF
    n_stream = n_t - n_keep

    nc = bacc.Bacc("TRN2", target_bir_lowering=False, debug=False,
                   num_devices=N_CORES)
    x = nc.dram_tensor("x", [NELEM], DT, kind="ExternalInput")
    y = nc.dram_tensor("y", [NELEM], DT, kind="ExternalOutput")

    def blk(dram, i):
        return dram[i * ch:(i + 1) * ch].rearrange("(p f) -> p f", f=tile_f)

    with tile.TileContext(nc) as tc:
        with tc.tile_pool(name="data", bufs=bufs) as data, \
             tc.tile_pool(name="keep", bufs=max(n_keep, 1)) as keepp, \
             tc.tile_pool(name="small", bufs=reps) as small, \
             tc.tile_pool(name="dram", bufs=1, space="DRAM") as dram:
          for _rep in range(reps):
            # ---------------- pass 1: local abs-max ----------------
            # the last n_keep tiles load into a dedicated pool and stay
            # resident so pass 2 skips re-reading them from HBM
            stats = small.tile([P, n_t], DT, tag="stats")
            kept = []
            for i in range(n_t):
                if i < n_stream:
                    t = data.tile([P, tile_f], DT, tag="blk")
                else:
                    t = keepp.tile([P, tile_f], DT, tag="keep")
                    kept.append(t)
                nc.sync.dma_start(out=t[:], in_=blk(x, i))
                nc.vector.tensor_reduce(out=stats[:, i:i + 1], in_=t[:],
                                        axis=mybir.AxisListType.X,
                                        op=A.max, apply_absolute_value=True)
            lmax = small.tile([P, 1], DT, tag="lmax")
            nc.vector.tensor_reduce(out=lmax[:], in_=stats[:],
                                    axis=mybir.AxisListType.X, op=A.max)
            amax = small.tile([P, 1], DT, tag="amax")
            nc.gpsimd.partition_all_reduce(amax[:], lmax[:], channels=P,
                                           reduce_op=bass_isa.ReduceOp.max)
            # zeros map to 1e-10 in the reference, so m >= 1e-10
            nc.vector.tensor_scalar(out=amax[:], in0=amax[:], scalar1=1e-10,
                                    scalar2=None, op0=A.max)

            # -------- all-reduce(max) of one scalar across 8 cores --------
            cc_in = dram.tile([1, 1], DT, tag="cc_in")
            gmax = small.tile([P, 1], DT, tag="gmax")
            nc.sync.dma_start(out=cc_in[:], in_=amax[0:1, 0:1])
            if cc == "ar":
                cc_out = dram.tile([1, 1], DT, tag="cc_out")
                nc.gpsimd.collective_compute(
                    "AllReduce", A.max,
                    replica_groups=[list(range(N_CORES))],
                    ins=[cc_in[:]], outs=[cc_out[:]],
                )
                gm1 = small.tile([1, 1], DT, tag="gm1")
                nc.sync.dma_start(out=gm1[:], in_=cc_out[:])
                nc.gpsimd.partition_broadcast(gmax[:], gm1[:])
            else:
                # AllGather (lower floor than AllReduce) + local max of the
                # 8 per-core maxima
                cc_out = dram.tile([N_CORES, 1], DT, tag="cc_out")
                nc.gpsimd.collective_compute(
                    "AllGather", A.bypass,
                    replica_groups=[list(range(N_CORES))],
                    ins=[cc_in[:]], outs=[cc_out[:]],
                )
                gm8 = small.tile([1, N_CORES], DT, tag="gm8")
                nc.sync.dma_start(
                    out=gm8[:], in_=cc_out[:].rearrange("r one -> one r"))
                gm1 = small.tile([1, 1], DT, tag="gm1")
                nc.vector.tensor_reduce(out=gm1[:], in_=gm8[:],
                                        axis=mybir.AxisListType.X, op=A.max)
                nc.gpsimd.partition_broadcast(gmax[:], gm1[:])

            # ------------- scales via exact bit arithmetic -------------
            bits = gmax[:].bitcast(DI)
            p_i = small.tile([P, 1], DI, tag="p_i")
            nc.vector.tensor_scalar(out=p_i[:], in0=bits, scalar1=0x7F800000,
                                    scalar2=None, op0=A.bitwise_and)
            s2i = small.tile([P, 1], DI, tag="s2i")
            nc.vector.tensor_scalar(out=s2i[:], in0=p_i[:], scalar1=6 << 23,
                                    scalar2=None, op0=A.subtract)
            s1i = small.tile([P, 1], DI, tag="s1i")
            nc.vector.tensor_scalar(out=s1i[:], in0=p_i[:], scalar1=254 << 23,
                                    scalar2=-1.0, op0=A.subtract, op1=A.mult)
            nc.vector.tensor_scalar(out=s1i[:], in0=s1i[:], scalar1=6 << 23,
                                    scalar2=None, op0=A.add)
            s1 = s1i[:].bitcast(DT)
            s2 = s2i[:].bitcast(DT)

            # ---------------- pass 2: quantize ----------------
            # kept tiles first: DVE has work immediately after the
            # collective while the streaming loads ramp back up
            def quantize(t):
                nc.vector.tensor_scalar(out=t[:], in0=t[:], scalar1=s1,
                                        scalar2=C_MAGIC,
                                        op0=A.mult, op1=A.add)
                eng = getattr(nc, clip_engine)
                eng.tensor_scalar(out=t[:], in0=t[:],
                                  scalar1=C_MAGIC + 127.0,
                                  scalar2=C_MAGIC - 128.0,
                                  op0=A.min, op1=A.max)
                nc.vector.tensor_scalar(out=t[:], in0=t[:], scalar1=-C_MAGIC,
                                        scalar2=s2, op0=A.add, op1=A.mult)

            for j, t in enumerate(kept):
                quantize(t)
                nc.sync.dma_start(out=blk(y, n_stream + j), in_=t[:])
            for i in range(n_stream):
                t = data.tile([P, tile_f], DT, tag="blk")
                nc.sync.dma_start(out=t[:], in_=blk(x, i))
                quantize(t)
                nc.sync.dma_start(out=blk(y, i), in_=t[:])

    nc.compile()
    return nc


def _get_nc(reps: int = 1, tile_f: int = TILE_F, bufs: int = BUFS,
            clip_engine: str = "vector", keep: int = KEEP, cc: str = "ar"):
    key = (reps, tile_f, bufs, clip_engine, keep, cc)
    if key not in _CACHE:
        _CACHE[key] = _build(reps, tile_f, bufs, clip_engine, keep, cc)
    return _CACHE[key]


def _get_fn():
    """Jitted 8-core executable, compiled once and reused across calls
    (run_bass_kernel_spmd would re-jit -> full ~25 s recompile per call)."""
    if "fn" in _CACHE:
        return _CACHE["fn"]
    import jax
    from jax.sharding import Mesh, NamedSharding, PartitionSpec
    from jax.experimental.shard_map import shard_map
    from concourse import bass2jax
    from concourse.bass2jax import _bass_exec_p, partition_id_tensor

    bass2jax.install_neuronx_cc_hook()
    nc = _get_nc()
    devices = jax.devices()[:N_CORES]
    mesh = Mesh(np.asarray(devices), ("core",))
    out_aval = jax.core.ShapedArray((NELEM,), np.float32)

    def _body(xa, ya):
        outs = _bass_exec_p.bind(
            xa, ya, partition_id_tensor(),
            out_avals=(out_aval,),
            in_names=("x", "y", nc.partition_id_tensor.name),
            out_names=("y",),
            lowering_input_output_aliases=(),
            sim_require_finite=True,
            sim_require_nnan=True,
            nc=nc,
        )
        return outs[0]

    fn = jax.jit(shard_map(
        _body, mesh=mesh,
        in_specs=(PartitionSpec("core"), PartitionSpec("core")),
        out_specs=PartitionSpec("core"), check_rep=False))
    sharding = NamedSharding(mesh, PartitionSpec("core"))
    # output operand buffer: materialized on device (not shipped over the
    # host link) and reused across calls -- it is never mutated since the
    # custom call's result is a fresh buffer
    import jax.numpy as jnp
    yd = jax.jit(lambda: jnp.zeros((N_CORES * NELEM,), jnp.float32),
                 out_shardings=sharding)()
    yd.block_until_ready()
    _CACHE["fn"] = (fn, sharding, yd)
    return _CACHE["fn"]


def kernel(x: np.ndarray) -> np.ndarray:
    import jax

    x = np.ascontiguousarray(np.asarray(x), dtype=np.float32)
    assert x.shape == (B, S, D), x.shape
    fn, sharding, yd = _get_fn()
    xd = jax.device_put(x.reshape(N_CORES * NELEM), sharding)
    out = np.asarray(fn(xd, yd))
    return out.reshape(B, S, D)

